# revision 28
# baseline (speedup 1.0000x reference)
"""EdgeConv (gather -> MLP -> segment_max) on 8 TRN2 NeuronCores via Bass/Tile.

Strategy
--------
Edges are sorted by destination (row) on the host and sharded contiguously
across the 8 cores at node granularity (no node's edges split cores, so no
cross-core combine is needed). Each core:

  * gathers x[row]/x[col] directly in TRANSPOSED orientation (features on
    partitions, edges on the free dim) with dma_gather(transpose=True) from a
    bf16 "pair token" table (256B tokens keep idx <= 25000 so int16 indices
    work).  The table is one flat buffer [zeros(128) | x.flat | zeros(192)]
    with two AP views: NORMAL rows u = [x[2u-2], x[2u-1]] and SHIFTED (offset
    64 elems) rows u = [x[2u-1], x[2u]].  Choosing the view per parity places
    x[col] on partitions 64:128 for both parities (even: shifted/idx=v>>1,
    odd: normal/idx=(v>>1)+1; misses read a guaranteed-zero row) and x[row]
    on partitions 0:64 (row parity is constant per CHUNK because nodes are
    ordered evens-then-odds and the section boundary is CHUNK-aligned).
  * x[col] needs two gathers (even cols / odd cols); their wanted halves are
    summed (exactly one term is nonzero) — all DVE ops partition-aligned.
  * MLP runs as bf16 matmuls with edges on the moving free dim:
    h.T[256,512] = W1.T @ feats.T (K=128 packed + K=32 edge_attr chunk),
    ReLU+bias on ACT, out.T[64,512] = W2.T @ h.T (K=256 in 2 chunks).
  * edges are grouped in 8-slot groups per node (padded with a repeated real
    edge), so a fixed-stride reduce-by-8 max on DVE produces per-group
    partials (L1).  Final per-node combine (avg 2.4 groups/node) + unpadding
    is done on the host from the L1/out.T slabs.
"""

import sys

for _p in ("/opt/trn_rl_repo",):
    if _p not in sys.path:
        sys.path.insert(0, _p)

import numpy as np
import ml_dtypes

N = 50000
E = 800000
D = 64
DE = 32
H = 256
NC = 8
TILE = 512
CHUNK = 4096
GRP = 4

_program_cache = {}


# --------------------------------------------------------------------------
# host-side preparation
# --------------------------------------------------------------------------

def _ceil_to(a, m):
    return (a + m - 1) // m * m


def _prep_core(order, row, col, cum, nlo, nhi):
    """Build the padded, grouped slot list for nodes [nlo, nhi).

    Returns dict with per-section slot arrays and node/group metadata.
    Slots are edge ids into the global edge list, -1 for dummy padding.
    """
    nodes = np.arange(nlo, nhi, dtype=np.int64)
    deg_all = cum[nodes + 1] - cum[nodes]
    out = {}
    for par in (0, 1):
        nsec = nodes[(nodes & 1) == par]
        dsec = cum[nsec + 1] - cum[nsec]
        present = nsec[dsec > 0]
        dp = (cum[present + 1] - cum[present]).astype(np.int64)
        ngrp = (dp + GRP - 1) // GRP
        nslots = ngrp * GRP
        total = int(nslots.sum())
        if total:
            starts = np.zeros(len(present) + 1, dtype=np.int64)
            np.cumsum(nslots, out=starts[1:])
            pos = np.arange(total, dtype=np.int64) - np.repeat(starts[:-1], nslots)
            node_rep = np.repeat(present, nslots)
            dpr = np.repeat(dp, nslots)
            eid = order[cum[node_rep] + np.minimum(pos, dpr - 1)]
        else:
            eid = np.empty(0, dtype=np.int64)
        out[par] = {
            "eid": eid,
            "present": present,
            "ngrp": ngrp,
        }
    return out


def _interleave_idx(wr, wa, wb, nslot):
    """Per-chunk contiguous [row | colA | colB] wrapped-idx blocks."""
    ccols_r = CHUNK // GRP // 16
    ccols = CHUNK // 16
    n_chunks = nslot // CHUNK
    out = np.empty((128, n_chunks * (ccols_r + 2 * ccols)), dtype=np.int16)
    w = ccols_r + 2 * ccols
    for ch in range(n_chunks):
        o = ch * w
        out[:, o : o + ccols_r] = wr[:, ch * ccols_r : (ch + 1) * ccols_r]
        out[:, o + ccols_r : o + ccols_r + ccols] = wa[:, ch * ccols : (ch + 1) * ccols]
        out[:, o + ccols_r + ccols : o + w] = wb[:, ch * ccols : (ch + 1) * ccols]
    return out


def _wrap_idx(idx_i16, nslot):
    """idx j -> [j%16, j//16], replicated to the 8 groups of 16 partitions."""
    w = nslot // 16
    a = idx_i16.reshape(w, 16).T  # [16, w]
    return np.tile(a, (8, 1)).copy()  # [128, w]


def _host_prep(x, edge_index, edge_attr):
    row = np.asarray(edge_index[0]).astype(np.int64)
    col = np.asarray(edge_index[1]).astype(np.int64)

    order = np.argsort(row, kind="stable")
    deg = np.bincount(row, minlength=N).astype(np.int64)
    cum = np.zeros(N + 1, dtype=np.int64)
    np.cumsum(deg, out=cum[1:])

    targets = np.arange(1, NC) * (E // NC)
    bounds = np.searchsorted(cum, targets, side="left")
    node_lo = np.concatenate([[0], bounds]).astype(np.int64)
    node_hi = np.concatenate([bounds, [N]]).astype(np.int64)

    cores = [
        _prep_core(order, row, col, cum, node_lo[c], node_hi[c]) for c in range(NC)
    ]

    sev = _ceil_to(max(len(c[0]["eid"]) for c in cores), TILE)
    sodd = max(len(c[1]["eid"]) for c in cores)
    nslot = _ceil_to(sev + sodd, CHUNK)

    ea_bf = edge_attr.astype(ml_dtypes.bfloat16)

    per_core = []
    for c in range(NC):
        eid = np.full(nslot, -1, dtype=np.int64)
        e0 = cores[c][0]["eid"]
        e1 = cores[c][1]["eid"]
        eid[: len(e0)] = e0
        eid[sev : sev + len(e1)] = e1
        real = eid >= 0
        eidc = np.where(real, eid, 0)

        r = row[eidc]
        cl = col[eidc]

        # row idx at GROUP granularity (all 8 slots of a group share the row)
        rg = r[::GRP]
        real_g = real[::GRP]
        idx_row = np.where(real_g, (rg >> 1) + 1, 0).astype(np.int16)
        cpar = cl & 1
        # even cols -> SHIFTED view, idx v>>1 (miss/dummy: 25000 = zero row)
        idx_ca = np.where(real & (cpar == 0), cl >> 1, N // 2).astype(np.int16)
        # odd cols -> NORMAL view, idx (v>>1)+1 (miss/dummy: 0 = zero row)
        idx_cb = np.where(real & (cpar == 1), (cl >> 1) + 1, 0).astype(np.int16)

        ea_t = np.ascontiguousarray(ea_bf[eidc].T)  # [32, nslot]
        ea_t[:, ~real] = 0

        # group metadata for host-side final combine
        g0, g1 = cores[c][0], cores[c][1]
        ngrp0 = int(len(e0) // GRP)
        ngrp1 = int(len(e1) // GRP)
        per_core.append(
            {
                "idx_all": _interleave_idx(
                    _wrap_idx(idx_row, nslot // GRP),
                    _wrap_idx(idx_ca, nslot),
                    _wrap_idx(idx_cb, nslot),
                    nslot,
                ),
                "ea_t": ea_t,
                "eid": eid,
                "real": real,
                # real group ranges: [0, ngrp0) and [sev/GRP, sev/GRP+ngrp1)
                "ngrp0": ngrp0,
                "ngrp1": ngrp1,
                "present": np.concatenate([g0["present"], g1["present"]]),
                "c1": np.concatenate([g0["ngrp"], g1["ngrp"]]).astype(np.int64),
            }
        )

    # flat pair-token table: [zeros(128) | x.flat | zeros(192)] as [25002, 128]
    xp = np.zeros((N // 2 + 2, 2 * D), dtype=ml_dtypes.bfloat16)
    xp[1 : N // 2 + 1, :] = x.astype(ml_dtypes.bfloat16).reshape(N // 2, 2 * D)

    return per_core, xp, nslot, sev


# --------------------------------------------------------------------------
# device program
# --------------------------------------------------------------------------

def _build_program(nslot, sev, b1_zero):
    import concourse.bacc as bacc
    import concourse.bass as bass
    import concourse.mybir as mybir
    import concourse.tile as tile
    from concourse._compat import get_trn_type

    bf16 = mybir.dt.bfloat16
    f32 = mybir.dt.float32
    i16 = mybir.dt.int16
    AF = mybir.ActivationFunctionType
    ts = bass.ts

    n_chunks = nslot // CHUNK
    pairs_per_chunk = CHUNK // (2 * TILE)
    wcols = nslot // 16       # wrapped idx columns (col gathers)
    wcols_r = nslot // GRP // 16  # wrapped idx columns (row gather, per group)
    ccols = CHUNK // 16
    ccols_r = CHUNK // GRP // 16
    gpc = CHUNK // GRP        # row-gather indices per chunk

    nc = bacc.Bacc(get_trn_type() or "TRN2", target_bir_lowering=False, debug=False)

    xpairs = nc.dram_tensor("xpairs", [N // 2 + 2, 2 * D], bf16, kind="ExternalInput")
    ea_t = nc.dram_tensor("ea_t", [DE, nslot], bf16, kind="ExternalInput")
    idx_all = nc.dram_tensor(
        "idx_all", [128, wcols_r + 2 * wcols], i16, kind="ExternalInput"
    )
    w1ab = nc.dram_tensor("w1ab", [2 * D, H], bf16, kind="ExternalInput")
    w1c = nc.dram_tensor("w1c", [DE, H], bf16, kind="ExternalInput")
    w2a = nc.dram_tensor("w2a", [128, D], bf16, kind="ExternalInput")
    w2b = nc.dram_tensor("w2b", [128, D], bf16, kind="ExternalInput")
    b1c = nc.dram_tensor("b1c", [128, 2], f32, kind="ExternalInput")

    # paired layout: partitions 0:64 = even tile of the pair, 64:128 = odd
    outT = nc.dram_tensor("outT", [128, nslot // 2], bf16, kind="ExternalOutput")
    l1 = nc.dram_tensor("l1", [128, nslot // GRP // 2], bf16, kind="ExternalOutput")

    ntok = N // 2 + 1  # gatherable rows per view
    xp_flat = xpairs[:, :].rearrange("a b -> (a b)")
    view_n = xp_flat[0 : ntok * 2 * D].rearrange("(a b) -> a b", b=2 * D)
    view_s = xp_flat[D : D + ntok * 2 * D].rearrange("(a b) -> a b", b=2 * D)

    with tile.TileContext(nc) as tc:
        with (
            tc.tile_pool(name="consts", bufs=1) as consts,
            tc.tile_pool(name="gather", bufs=3) as gpool,
            tc.tile_pool(name="idx", bufs=3) as ipool,
            tc.tile_pool(name="ea", bufs=3) as eapool,
            tc.tile_pool(name="feat", bufs=3) as fpool,
            tc.tile_pool(name="hsb", bufs=3) as hpool,
            tc.tile_pool(name="ot", bufs=3) as opool,
            tc.tile_pool(name="psum_h", bufs=2, space="PSUM") as ph,
            tc.tile_pool(name="psum_o", bufs=2, space="PSUM") as po,
        ):
            w1ab_sb = consts.tile([2 * D, H], bf16)
            w1c_sb = consts.tile([DE, H], bf16)
            w2a_sb = consts.tile([128, D], bf16)
            w2b_sb = consts.tile([128, D], bf16)
            b1_sb = consts.tile([128, 2], f32)
            nc.sync.dma_start(out=w1ab_sb[:], in_=w1ab[:])
            nc.sync.dma_start(out=w1c_sb[:], in_=w1c[:])
            nc.sync.dma_start(out=w2a_sb[:], in_=w2a[:])
            nc.sync.dma_start(out=w2b_sb[:], in_=w2b[:])
            nc.sync.dma_start(out=b1_sb[:], in_=b1c[:])

            for ch in range(n_chunks):
                ix = ipool.tile([128, ccols_r + 2 * ccols], i16, tag="ix")
                nc.sync.dma_start(
                    out=ix[:], in_=idx_all[:, ts(ch, ccols_r + 2 * ccols)]
                )
                ir = ix[:, 0:ccols_r]
                ia = ix[:, ccols_r : ccols_r + ccols]
                ib = ix[:, ccols_r + ccols :]

                lo, hi = ch * CHUNK, (ch + 1) * CHUNK
                # groups in [lo, min(sev,hi)) use view_n; rest view_s
                n_even_g = (min(max(sev - lo, 0), CHUNK)) // GRP

                gr = gpool.tile([128, 1, gpc], bf16, tag="gr")
                ga = gpool.tile([128, 1, CHUNK], bf16, tag="ga")
                gb = gpool.tile([128, 1, CHUNK], bf16, tag="gb")
                if 0 < n_even_g < gpc:
                    assert n_even_g % 16 == 0
                    nc.gpsimd.dma_gather(
                        gr[:, :, 0:n_even_g], view_n, ir[:, 0 : n_even_g // 16],
                        n_even_g, n_even_g, 2 * D,
                        transpose=True, single_packet=False)
                    nc.gpsimd.dma_gather(
                        gr[:, :, n_even_g:gpc], view_s,
                        ir[:, n_even_g // 16 : ccols_r],
                        gpc - n_even_g, gpc - n_even_g, 2 * D,
                        transpose=True, single_packet=False)
                else:
                    row_view = view_n if n_even_g == gpc else view_s
                    nc.gpsimd.dma_gather(gr[:], row_view, ir, gpc, gpc, 2 * D,
                                         transpose=True, single_packet=False)
                nc.gpsimd.dma_gather(ga[:], view_s, ia, CHUNK, CHUNK, 2 * D,
                                     transpose=True, single_packet=False)
                nc.gpsimd.dma_gather(gb[:], view_n, ib, CHUNK, CHUNK, 2 * D,
                                     transpose=True, single_packet=False)

                eat = eapool.tile([DE, CHUNK], bf16, tag="eat")
                nc.sync.dma_start(out=eat[:], in_=ea_t[:, ts(ch, CHUNK)])

                oc = opool.tile([128, CHUNK // 2], bf16, tag="oc")
                lc = opool.tile([128, CHUNK // GRP // 2], bf16, tag="lc")

                for ip in range(pairs_per_chunk):
                    hs = []
                    for half in range(2):
                        it = 2 * ip + half
                        sl = ts(it, TILE)
                        gsl = ts(it, TILE // GRP)

                        f = fpool.tile([2 * D, TILE], bf16, tag="f")
                        nc.vector.tensor_copy(
                            out=f[0:D, :].rearrange("p (g e) -> p g e", e=GRP),
                            in_=gr[0:D, 0, gsl, None].to_broadcast(
                                [D, TILE // GRP, GRP]
                            ),
                        )
                        nc.vector.tensor_tensor(
                            out=f[D : 2 * D, :],
                            in0=ga[D : 2 * D, 0, sl],
                            in1=gb[D : 2 * D, 0, sl],
                            op=mybir.AluOpType.add,
                        )

                        hp = ph.tile([2 * D, 2 * TILE], f32, tag="hp")
                        h1p = hp[:, 0:TILE]
                        h2p = hp[:, TILE : 2 * TILE]
                        nc.tensor.matmul(h1p, w1ab_sb[:, 0:128], f[:],
                                         start=True, stop=False)
                        nc.tensor.matmul(h1p, w1c_sb[:, 0:128], eat[:, sl],
                                         start=False, stop=True)
                        nc.tensor.matmul(h2p, w1ab_sb[:, 128:256], f[:],
                                         start=True, stop=False)
                        nc.tensor.matmul(h2p, w1c_sb[:, 128:256], eat[:, sl],
                                         start=False, stop=True)

                        h12 = hpool.tile([2 * D, 2 * TILE], bf16, tag="h12")
                        if b1_zero:
                            nc.scalar.activation(h12[:], hp[:], AF.Relu)
                        else:
                            nc.scalar.activation(h12[:, 0:TILE], h1p, AF.Relu,
                                                 bias=b1_sb[:, 0:1])
                            nc.scalar.activation(h12[:, TILE : 2 * TILE], h2p,
                                                 AF.Relu, bias=b1_sb[:, 1:2])
                        hs.append((h12[:, 0:TILE], h12[:, TILE : 2 * TILE]))

                    op_ = po.tile([128, TILE], f32, tag="op")
                    for half in range(2):
                        h1, h2 = hs[half]
                        psl = op_[64 * half : 64 * half + 64, :]
                        tp = (0, 64 * half)
                        nc.tensor.matmul(psl, w2a_sb[:], h1[:], start=True,
                                         stop=False, tile_position=tp)
                        nc.tensor.matmul(psl, w2b_sb[:], h2[:], start=False,
                                         stop=True, tile_position=tp)

                    psl_all = ts(ip, TILE)
                    nc.vector.tensor_copy(out=oc[:, psl_all], in_=op_[:])

                nc.vector.reduce_max(
                    out=lc[:],
                    in_=oc[:, :].rearrange("p (g e) -> p g e", e=GRP),
                    axis=mybir.AxisListType.X,
                )
                nc.sync.dma_start(out=outT[:, ts(ch, CHUNK // 2)], in_=oc[:])
                nc.sync.dma_start(out=l1[:, ts(ch, CHUNK // GRP // 2)], in_=lc[:])

    nc.compile()
    return nc


# --------------------------------------------------------------------------
# entry point
# --------------------------------------------------------------------------

def kernel(x, edge_index, edge_attr, W1, b1, W2, b2):
    from concourse.bass_utils import run_bass_kernel_spmd

    if not hasattr(kernel, "run_kwargs"):
        kernel.run_kwargs = {}

    x = np.asarray(x)
    edge_attr = np.asarray(edge_attr)
    W1 = np.asarray(W1, dtype=np.float32)
    b1 = np.asarray(b1, dtype=np.float32)
    W2 = np.asarray(W2, dtype=np.float32)
    b2 = np.asarray(b2, dtype=np.float32)

    per_core, xp, nslot, sev = _host_prep(x, edge_index, edge_attr)

    b1_zero = bool(np.all(b1 == 0.0))
    key = (nslot, sev, b1_zero)
    if key not in _program_cache:
        _program_cache[key] = _build_program(nslot, sev, b1_zero)
    nc = _program_cache[key]

    bf = ml_dtypes.bfloat16
    w1ab = np.ascontiguousarray(W1[: 2 * D].astype(bf))
    w1c = np.ascontiguousarray(W1[2 * D :].astype(bf))
    w2a = np.ascontiguousarray(W2[:128].astype(bf))
    w2b = np.ascontiguousarray(W2[128:].astype(bf))
    b1c = np.ascontiguousarray(b1.reshape(2, 128).T.astype(np.float32))

    in_maps = []
    for c in range(NC):
        pc = per_core[c]
        in_maps.append(
            {
                "xpairs": xp,
                "ea_t": pc["ea_t"],
                "idx_all": pc["idx_all"],
                "w1ab": w1ab,
                "w1c": w1c,
                "w2a": w2a,
                "w2b": w2b,
                "b1c": b1c,
            }
        )

    res = run_bass_kernel_spmd(
        nc, in_maps, core_ids=list(range(NC)), **kernel.run_kwargs
    )
    kernel.last_results = res

    return _postprocess(per_core, res.results, nslot, sev, b2)


def _postprocess(per_core, results, nslot, sev, b2):
    out_full = np.empty((E, D), dtype=np.float32)
    agg = np.zeros((N, D), dtype=np.float32)
    b2f = b2.astype(np.float32)
    for c in range(NC):
        pc = per_core[c]
        r = results[c]
        # unpack paired layout [128, nslot/2] -> [nslot, D]
        vals = (
            np.asarray(r["outT"]).astype(np.float32)
            .reshape(2, D, nslot // 1024, TILE)
            .transpose(2, 0, 3, 1)
            .reshape(nslot, D)
        )
        real = pc["real"]
        out_full[pc["eid"][real]] = vals[real] + b2f

        l1v = (
            np.asarray(r["l1"]).astype(np.float32)
            .reshape(2, D, nslot // (2 * TILE), TILE // GRP)
            .transpose(2, 0, 3, 1)
            .reshape(nslot // GRP, D)
        )
        ngrp0, ngrp1 = pc["ngrp0"], pc["ngrp1"]
        g_real = np.concatenate(
            [l1v[:ngrp0], l1v[sev // GRP : sev // GRP + ngrp1]], axis=0
        )
        if len(pc["present"]):
            starts = np.zeros(len(pc["c1"]), dtype=np.int64)
            np.cumsum(pc["c1"][:-1], out=starts[1:])
            seg = np.maximum.reduceat(g_real, starts, axis=0)
            agg[pc["present"]] = seg + b2f

    return agg, out_full
